# revision 16
# baseline (speedup 1.0000x reference)
"""Dense dot-product attention with key-length masking on 8 Trainium2 cores.

Problem: q,k,v [16, 2048, 128] fp32, valid_lens [16,1] int32.
  out = softmax(mask(q@k.T/sqrt(d))) @ v   (masked keys -> -1e6 before softmax)

v4 design:
- Flexible work packing: every core runs the same slot-size profile; a
  slot holds ONE (batch, q-half) unit's key-tile RANGE, and a unit may
  split across any slots/cores.  The host adds partial numerators /
  denominators, so packing is near perfect; per-tile masks make padding
  free.
- Slots come in TWO dtype classes, baked into the program:
  * fp8 slots hold tiles of LARGE-valid_len units (L >= 1024).  Their
    softmax is diffuse (max weight ~1-2%), so e4m3's 6% quantization is
    harmless.  E pairs [128,2,1024] fp8 drive DoubleRow matmuls (2 key
    tiles per pass = 2x PE).
  * fp16 slots hold small-L units (peaked softmax needs precision).
    Regular fp16 matmuls.
- E = exp(S/sqrt(d) - 2): the -2 shift cancels in softmax and keeps
  E < 240 (e4m3-safe for scores up to 7.5 sigma).
- exp runs on TWO engines: ACT (activation Exp) and DVE (Schraudolph
  bit trick: round(S*a + b) in uint8/int16, bitcast as e4m3/fp16;
  uint8/int16 conversion saturates underflow to +0.0; masked partitions
  get scale'=0, bias'=0 -> E=+0.0).
- NO on-device softmax reduction: all E pairs stream to DRAM and the
  host computes denominators from exactly the bytes the PE consumed.
- oT fp32->fp16 PSUM drain copies split between ACT and DVE.
- Input DMA on the SP queue in slot order; outputs kick on the idle
  Pool queue (Pool tensor math would contend for SBUF ports with DVE).
- HAM warm-up: dummy bf16 matmuls while the input DMAs stream.
"""

import math
import sys
import types

import numpy as np
import ml_dtypes

import concourse.bass as bass
import concourse.mybir as mybir
import concourse.tile as tile
from concourse.tile import add_dep_helper
from concourse import bacc
from concourse.bass_utils import run_bass_kernel_spmd

B, Q, K, D = 16, 2048, 2048, 128
NCORES = 8
QCH = 1024         # queries per work unit
KT = K // 128      # max key tiles per unit
MM_N = 512         # moving-operand free dim per matmul
SCALE = 1.0 / math.sqrt(D)
SHIFT = 2.0        # exp(S*SCALE - SHIFT): cancels in softmax, keeps E < 240
L8 = 1024          # units with L >= L8 use fp8 E (diffuse softmax)
NEG_BIAS = -30.0   # exp(-30) ~ 1e-13 -> flushes to +0
WARMUP_MMS = 7     # dummy matmuls to lift the PE HAM clock-gate
LOG2E = math.log2(math.e)
SCH_MUL = 1024.0 * LOG2E                             # fp16 Schraudolph
SCH_BIAS = 15360.0 - 1024.0 * LOG2E * SHIFT - 45.0
SCH8_MUL = 8.0 * LOG2E                               # e4m3 Schraudolph
SCH8_BIAS = 56.0 - 8.0 * LOG2E * SHIFT - 0.351
ACT_TILES = 18                    # exp tiles on ACT (rest on DVE)
ACT_COPIES = 3                    # oT drain copies done by ACT (rest DVE)

F32 = mybir.dt.float32
F16 = mybir.dt.float16
BF16 = mybir.dt.bfloat16
F8 = mybir.dt.float8e4
U8 = mybir.dt.uint8
I16 = mybir.dt.int16
FN = ml_dtypes.float8_e4m3fn


def _install_hook_stub():
    """bass_utils' axon trace path imports antenv.axon_hooks, which is not
    shipped in this container.  Provide a no-op stub so an ambient
    BASS_TRACE=1 doesn't crash; test harnesses may overwrite the hook."""
    if "antenv.axon_hooks" in sys.modules:
        return
    mod = types.ModuleType("antenv.axon_hooks")
    _hook = [None]
    mod.set_axon_ntff_profile_hook = lambda h: _hook.__setitem__(0, h)
    mod.get_axon_ntff_profile_hook = lambda: _hook[0]
    sys.modules["antenv.axon_hooks"] = mod


_install_hook_stub()

_build_cache = {}
last_result = None  # BassKernelResults of the most recent run (for harnesses)


# ---------------------------------------------------------------- planning

def _pack(profile, units):
    """Greedy: place unit needs into 8x profile slot inventory, splitting
    freely; returns list of (unit, off, cnt, size_class) pieces or None."""
    inv = []
    for s in profile:
        inv += [s] * NCORES
    inv.sort(reverse=True)
    pieces = []
    order = sorted(range(len(units)), key=lambda o: -units[o][1])
    for o in order:
        u, n = units[o]
        rem, off = n, 0
        while rem > 0:
            if not inv:
                return None
            le = [s for s in inv if s <= rem]
            s = max(le) if le else min(inv)
            inv.remove(s)
            take = min(s, rem)
            pieces.append((u, off, take, s))
            off += take
            rem -= take
    return pieces


def _best_profile(units):
    """units: list of (unit_idx, need).  Returns (profile, pieces)."""
    if not units:
        return (), []
    total = sum(n for _, n in units)
    tmin = (total + NCORES - 1) // NCORES
    from itertools import combinations_with_replacement as cwr
    best = None
    for T in range(tmin, tmin + 4):
        for m in range(1, 6):
            for prof in cwr(range(1, KT + 1), m):
                if sum(prof) != T:
                    continue
                pieces = _pack(prof, units)
                if pieces is None:
                    continue
                key = (T, m)
                if best is None or key < best[0]:
                    best = (key, tuple(sorted(prof, reverse=True)), pieces)
        if best is not None and best[0][0] == T:
            break
    return best[1], best[2]


def _plan(need, fp8_unit):
    """Returns (slots, cls8, assign): slot sizes in execution order,
    per-slot fp8 flag, and assign[core][slot] = (unit|None, off, cnt)."""
    units8 = [(u, n) for u, n in enumerate(need) if fp8_unit[u]]
    units16 = [(u, n) for u, n in enumerate(need) if not fp8_unit[u]]
    prof8, pieces8 = _best_profile(units8)
    prof16, pieces16 = _best_profile(units16)
    # execution order: all slots ascending by size, smallest last (tail)
    tagged = [(s, True) for s in prof8] + [(s, False) for s in prof16]
    tagged.sort(key=lambda x: x[0])
    order = tagged[1:] + [tagged[0]] if len(tagged) >= 2 else tagged
    slots = tuple(s for s, _ in order)
    cls8 = tuple(c for _, c in order)
    cells = {}   # (size, is8) -> list of (core, slot_idx)
    for j, (s, c) in enumerate(zip(slots, cls8)):
        cells.setdefault((s, c), [])
        for co in range(NCORES):
            cells[(s, c)].append((co, j))
    assign = [[None] * len(slots) for _ in range(NCORES)]
    for pieces, is8 in ((pieces8, True), (pieces16, False)):
        for (u, off, cnt, s) in pieces:
            co, j = cells[(s, is8)].pop()
            assign[co][j] = (u, off, cnt)
    for co in range(NCORES):
        for j in range(len(slots)):
            if assign[co][j] is None:
                assign[co][j] = (None, 0, 0)
    return slots, cls8, assign


def _ownership(slots):
    """Per slot: which exp tiles run on DVE (True) vs ACT (False).
    The final (tail) slot is DVE so its exp chains with the drain copy."""
    T = sum(slots)
    dve_total = T - ACT_TILES
    owner = []
    for j, t in enumerate(slots):
        if j == len(slots) - 1:
            owner.append(tuple(True for _ in range(t)))
            continue
        d = int(round(t * dve_total / max(T, 1)))
        d = min(t, max(0, d))
        dve_pos = set()
        if d > 0:
            for i in range(d):
                dve_pos.add(int((i + 0.5) * t / d))
        owner.append(tuple(i in dve_pos for i in range(t)))
    return tuple(owner)


# ---------------------------------------------------------------- build

def _build(slots, cls8, owner):
    nc = bacc.Bacc(num_devices=NCORES)
    NS = len(slots)
    T = sum(slots)
    npairs = [(t + 1) // 2 for t in slots]
    soff = [sum(slots[:j]) for j in range(NS)]

    qT = nc.declare_dram_parameter("qT", [NS, D, QCH], F16, isOutput=False)
    kts = [
        nc.declare_dram_parameter(f"kt{j}", [128, slots[j] * 128], F16,
                                  isOutput=False)
        for j in range(NS)
    ]
    vs = [
        nc.declare_dram_parameter(f"v{j}", [128, npairs[j] * 2, 128],
                                  F8 if cls8[j] else F16, isOutput=False)
        for j in range(NS)
    ]
    # per-slot cols [4*soff, 4*soff+4*t): [sc_a | bi_a | sc_d | bi_d]
    sb = nc.declare_dram_parameter("sb", [128, 4 * T], F32, isOutput=False)
    oT = nc.declare_dram_parameter("oT", [NS, D, QCH], F16, isOutput=True)
    n8 = sum(npairs[j] for j in range(NS) if cls8[j])
    n16 = sum(npairs[j] for j in range(NS) if not cls8[j])
    ep8 = nc.declare_dram_parameter("ep8", [max(n8, 1), 128, 2, QCH], F8,
                                    isOutput=True)
    ep16 = nc.declare_dram_parameter("ep16", [max(n16, 1), 128, 2, QCH], F16,
                                     isOutput=True)

    with tile.TileContext(nc) as tc:
        with (
            tc.tile_pool(name="consts", bufs=1) as consts,
            tc.tile_pool(name="inputs", bufs=2) as inpool,
            tc.tile_pool(name="epool", bufs=max(npairs) + 4) as epool,
            tc.tile_pool(name="osb", bufs=2) as opool,
            tc.tile_pool(name="sps", bufs=3, space="PSUM") as pspool,
            tc.tile_pool(name="oacc", bufs=1, space="PSUM") as psacc,
        ):
            sb_sb = consts.tile([128, 4 * T], F32)

            # --- HAM warm-up: dummy bf16 matmuls while input DMAs stream ---
            wsrc = consts.tile([128, MM_N], BF16)
            nc.vector.memset(wsrc[:], 1.0)
            for w in range(WARMUP_MMS):
                if w % 2 == 0:
                    wps = pspool.tile([128, QCH], F32, tag="s")
                nc.tensor.matmul(
                    wps[:, (w % 2) * MM_N : (w % 2) * MM_N + MM_N],
                    wsrc[:, :128],
                    wsrc[:],
                    start=True,
                    stop=True,
                    skip_group_check=True,
                )

            ncopy = [0]
            i8 = [0]
            i16 = [0]
            for s in range(NS):
                t = slots[s]
                np_s = npairs[s]
                is8 = cls8[s]
                ow = owner[s]
                edt = F8 if is8 else F16
                qT_sb = inpool.tile([128, QCH], F16, tag="qT")
                kt_sb = inpool.tile([128, t * 128], F16, tag="kt")
                v_sb = inpool.tile([128, np_s * 2, 128], edt, tag="v")
                # kt first (first S matmul needs it), then qT halves, then v;
                # all on the SP queue so program order = priority order
                ktcols = t * 128
                piece = 2048
                npc = (ktcols + piece - 1) // piece
                for j in range(npc):
                    lo, hi = j * piece, min(ktcols, (j + 1) * piece)
                    nc.sync.dma_start(out=kt_sb[:, lo:hi], in_=kts[s][:, lo:hi])
                nq = 2 if s == 0 else 1
                for j in range(nq):
                    nc.sync.dma_start(
                        out=qT_sb[:, bass.ts(j, QCH // nq)],
                        in_=qT[s][:, bass.ts(j, QCH // nq)],
                    )
                nc.sync.dma_start(out=v_sb[:, :, :], in_=vs[s][:, :, :])
                if s == 0:
                    # masks land after slot0's inputs, before the first exp
                    nc.sync.dma_start(out=sb_sb[:], in_=sb[:])

                o_ps = psacc.tile([128, QCH], F32, tag="o")
                epair = None
                for i in range(t):
                    m = i // 2
                    if i % 2 == 0:
                        epair = epool.tile([128, 2, QCH], edt, tag="e")
                    s_ps = pspool.tile([128, QCH], F32, tag="s")
                    for h in range(QCH // MM_N):
                        nc.tensor.matmul(
                            s_ps[:, bass.ts(h, MM_N)],
                            kt_sb[:, bass.ts(i, 128)],
                            qT_sb[:, bass.ts(h, MM_N)],
                            start=True,
                            stop=True,
                        )
                    col = 4 * soff[s]
                    if ow[i]:
                        sc_ap = sb_sb[:, col + 2 * t + i : col + 2 * t + i + 1]
                        bi_ap = sb_sb[:, col + 3 * t + i : col + 3 * t + i + 1]
                        nc.vector.tensor_scalar(
                            epair[:, i % 2, :].bitcast(U8 if is8 else I16),
                            s_ps[:], sc_ap, bi_ap,
                            mybir.AluOpType.mult, mybir.AluOpType.add,
                        )
                    else:
                        sc_ap = sb_sb[:, col + i : col + i + 1]
                        bi_ap = sb_sb[:, col + t + i : col + t + i + 1]
                        parts = (
                            [bass.ts(p, MM_N) for p in range(2)]
                            if (s == 0 and i == 0)
                            else [slice(None)]
                        )
                        for pr in parts:
                            nc.scalar.activation(
                                epair[:, i % 2, pr],
                                s_ps[:, pr],
                                mybir.ActivationFunctionType.Exp,
                                bias=bi_ap,
                                scale=sc_ap,
                            )
                    last = i == t - 1
                    if is8 and i % 2 == 1:
                        # full fp8 pair -> DoubleRow matmul (2 key tiles/pass)
                        for h in range(QCH // MM_N):
                            nc.tensor.matmul(
                                o_ps[:, bass.ts(h, MM_N)],
                                v_sb[:, 2 * m : 2 * m + 2, :],
                                epair[:, :, bass.ts(h, MM_N)],
                                start=(m == 0),
                                stop=last,
                                perf_mode=mybir.MatmulPerfMode.DoubleRow,
                            )
                    elif is8 and last:
                        # odd fp8 tail: single regular fp8 matmul
                        for h in range(QCH // MM_N):
                            nc.tensor.matmul(
                                o_ps[:, bass.ts(h, MM_N)],
                                v_sb[:, 2 * m, :],
                                epair[:, 0, bass.ts(h, MM_N)],
                                start=(i == 0),
                                stop=True,
                            )
                    elif not is8:
                        # fp16: regular matmul per key tile
                        for h in range(QCH // MM_N):
                            nc.tensor.matmul(
                                o_ps[:, bass.ts(h, MM_N)],
                                v_sb[:, i, :],
                                epair[:, i % 2, bass.ts(h, MM_N)],
                                start=(i == 0),
                                stop=last,
                            )
                    if i % 2 == 1 or last:
                        if is8:
                            nc.gpsimd.dma_start(out=ep8[i8[0]],
                                                in_=epair[:, :, :])
                            i8[0] += 1
                        else:
                            nc.gpsimd.dma_start(out=ep16[i16[0]],
                                                in_=epair[:, :, :])
                            i16[0] += 1

                o_sb = opool.tile([128, QCH], F16, tag="osb")
                if ncopy[0] < ACT_COPIES:
                    nc.scalar.copy(o_sb[:], o_ps[:])
                else:
                    nc.vector.tensor_copy(o_sb[:], o_ps[:])
                ncopy[0] += 1
                nc.gpsimd.dma_start(out=oT[s], in_=o_sb[:])

    nc.compile()
    return nc


# ---------------------------------------------------------------- host

def kernel(q, k, v, valid_lens):
    q = np.ascontiguousarray(q, dtype=np.float32)
    k = np.ascontiguousarray(k, dtype=np.float32)
    v = np.ascontiguousarray(v, dtype=np.float32)
    L = np.asarray(valid_lens).reshape(-1).astype(np.int64)

    need_b = np.where(L == 0, KT, np.minimum(KT, (L + 127) // 128)).astype(np.int64)
    units = [(b, h) for b in range(B) for h in range(Q // QCH)]
    need = [int(need_b[b]) for b, h in units]
    fp8_unit = [int(L[b]) >= L8 for b, h in units]

    slots, cls8, assign = _plan(need, fp8_unit)
    owner = _ownership(slots)

    key = (slots, cls8, owner)
    if key not in _build_cache:
        _build_cache[key] = _build(slots, cls8, owner)
    nc = _build_cache[key]

    qh = q.astype(np.float16)
    kh = k.astype(np.float16)
    vh = v.astype(np.float16)
    v8 = v.astype(FN)

    NS = len(slots)
    T = sum(slots)
    npairs = [(t + 1) // 2 for t in slots]
    soff = [sum(slots[:j]) for j in range(NS)]

    in_maps = []
    for c in range(NCORES):
        qT_arr = np.zeros((NS, D, QCH), np.float16)
        sb_arr = np.zeros((128, 4 * T), np.float32)
        im = {"qT": qT_arr, "sb": sb_arr}
        for j in range(NS):
            t = slots[j]
            is8 = cls8[j]
            kt = np.zeros((128, t * 128), np.float16)
            vv = np.zeros((128, npairs[j] * 2, 128), FN if is8 else np.float16)
            u, off, cnt = assign[c][j]
            col = 4 * soff[j]
            if u is not None and cnt > 0:
                b, h = units[u]
                lb = int(L[b])
                qT_arr[j] = qh[b, h * QCH : (h + 1) * QCH].T
                kt[:, : cnt * 128] = kh[b, off * 128 : (off + cnt) * 128].T
                vsl = (v8 if is8 else vh)[b, off * 128 : (off + cnt) * 128]
                for i in range(cnt):
                    vv[:, i, :] = vsl[i * 128 : (i + 1) * 128]
                kidx = np.arange(128)
                smul = SCH8_MUL if is8 else SCH_MUL
                sbia = SCH8_BIAS if is8 else SCH_BIAS
                one = 56.0 if is8 else 15360.0
                for i in range(cnt):
                    base = (off + i) * 128
                    if lb == 0:
                        sb_arr[:, col + 3 * t + i] = one   # E = 1.0 uniform
                    else:
                        m = (base + kidx < lb).astype(np.float32)
                        sb_arr[:, col + i] = m * np.float32(SCALE)
                        sb_arr[:, col + t + i] = np.where(
                            m > 0, np.float32(-SHIFT), np.float32(NEG_BIAS))
                        sb_arr[:, col + 2 * t + i] = m * np.float32(SCALE * smul)
                        sb_arr[:, col + 3 * t + i] = m * np.float32(sbia)
            for i in range(cnt, t):
                sb_arr[:, col + t + i] = np.float32(NEG_BIAS)  # ACT padding
                # DVE padding: sc=0, bi=0 -> +0.0
            im[f"kt{j}"] = kt
            im[f"v{j}"] = vv
        in_maps.append(im)

    res = run_bass_kernel_spmd(nc, in_maps, list(range(NCORES)))
    global last_result
    last_result = res

    num = [np.zeros((D, QCH), np.float32) for _ in range(len(units))]
    den = [np.zeros((QCH,), np.float32) for _ in range(len(units))]
    # replicate the build's ep8/ep16 stream ordering: (slot, pair) -> index
    pair_idx = {}
    c8 = c16 = 0
    for j in range(NS):
        for m in range(npairs[j]):
            if cls8[j]:
                pair_idx[(j, m)] = (True, c8)
                c8 += 1
            else:
                pair_idx[(j, m)] = (False, c16)
                c16 += 1
    for c in range(NCORES):
        r = res.results[c]
        e8v = r["ep8"]
        e8v = (e8v.view(FN) if e8v.dtype != FN else e8v).astype(np.float32)
        e16v = r["ep16"].astype(np.float32)
        for j in range(NS):
            u, off, cnt = assign[c][j]
            if u is None or cnt == 0:
                continue
            num[u] += r["oT"][j].astype(np.float32)
            for i in range(cnt):
                isf8, idx = pair_idx[(j, i // 2)]
                arr = e8v if isf8 else e16v
                den[u] += arr[idx, :, i % 2, :].sum(axis=0)

    out = np.empty((B, Q, D), np.float32)
    for ui, (b, h) in enumerate(units):
        out[b, h * QCH : (h + 1) * QCH] = (num[ui] / den[ui][None, :]).T
    return out


# revision 17
# speedup vs baseline: 1.1104x; 1.1104x over previous
"""Dense dot-product attention with key-length masking on 8 Trainium2 cores.

Problem: q,k,v [16, 2048, 128] fp32, valid_lens [16,1] int32.
  out = softmax(mask(q@k.T/sqrt(d))) @ v   (masked keys -> -1e6 before softmax)

v4 design:
- Flexible work packing: every core runs the same slot-size profile; a
  slot holds ONE (batch, q-half) unit's key-tile RANGE, and a unit may
  split across any slots/cores.  The host adds partial numerators /
  denominators, so packing is near perfect; per-tile masks make padding
  free.
- Slots come in TWO dtype classes, baked into the program:
  * fp8 slots hold tiles of LARGE-valid_len units (L >= 1024).  Their
    softmax is diffuse (max weight ~1-2%), so e4m3's 6% quantization is
    harmless.  E pairs [128,2,1024] fp8 drive DoubleRow matmuls (2 key
    tiles per pass = 2x PE).
  * fp16 slots hold small-L units (peaked softmax needs precision).
    Regular fp16 matmuls.
- E = exp(S/sqrt(d) - 2): the -2 shift cancels in softmax and keeps
  E < 240 (e4m3-safe for scores up to 7.5 sigma).
- exp runs on TWO engines: ACT (activation Exp) and DVE (Schraudolph
  bit trick: round(S*a + b) in uint8/int16, bitcast as e4m3/fp16;
  uint8/int16 conversion saturates underflow to +0.0; masked partitions
  get scale'=0, bias'=0 -> E=+0.0).
- NO on-device softmax reduction: all E pairs stream to DRAM and the
  host computes denominators from exactly the bytes the PE consumed.
- oT fp32->fp16 PSUM drain copies split between ACT and DVE.
- Input DMA on the SP queue in slot order; outputs kick on the idle
  Pool queue (Pool tensor math would contend for SBUF ports with DVE).
- HAM warm-up: dummy bf16 matmuls while the input DMAs stream.
"""

import math
import sys
import types

import numpy as np
import ml_dtypes

import concourse.bass as bass
import concourse.mybir as mybir
import concourse.tile as tile
from concourse.tile import add_dep_helper
from concourse import bacc
from concourse.bass_utils import run_bass_kernel_spmd

B, Q, K, D = 16, 2048, 2048, 128
NCORES = 8
QCH = 1024         # queries per work unit
KT = K // 128      # max key tiles per unit
MM_N = 512         # moving-operand free dim per matmul
SCALE = 1.0 / math.sqrt(D)
SHIFT = 2.0        # exp(S*SCALE - SHIFT): cancels in softmax, keeps E < 240
L8 = 1024          # units with L >= L8 use fp8 E (diffuse softmax)
NEG_BIAS = -30.0   # exp(-30) ~ 1e-13 -> flushes to +0
WARMUP_MMS = 5     # dummy matmuls to lift the PE HAM clock-gate
LOG2E = math.log2(math.e)
SCH_MUL = 1024.0 * LOG2E                             # fp16 Schraudolph
SCH_BIAS = 15360.0 - 1024.0 * LOG2E * SHIFT - 45.0
SCH8_MUL = 8.0 * LOG2E                               # e4m3 Schraudolph
SCH8_BIAS = 56.0 - 8.0 * LOG2E * SHIFT - 0.351
ACT_TILES = 18                    # exp tiles on ACT (rest on DVE)
ACT_COPIES = 3                    # oT drain copies done by ACT (rest DVE)

F32 = mybir.dt.float32
F16 = mybir.dt.float16
BF16 = mybir.dt.bfloat16
F8 = mybir.dt.float8e4
U8 = mybir.dt.uint8
I16 = mybir.dt.int16
FN = ml_dtypes.float8_e4m3fn


def _install_hook_stub():
    """bass_utils' axon trace path imports antenv.axon_hooks, which is not
    shipped in this container.  Provide a no-op stub so an ambient
    BASS_TRACE=1 doesn't crash; test harnesses may overwrite the hook."""
    if "antenv.axon_hooks" in sys.modules:
        return
    mod = types.ModuleType("antenv.axon_hooks")
    _hook = [None]
    mod.set_axon_ntff_profile_hook = lambda h: _hook.__setitem__(0, h)
    mod.get_axon_ntff_profile_hook = lambda: _hook[0]
    sys.modules["antenv.axon_hooks"] = mod


_install_hook_stub()

_build_cache = {}
last_result = None  # BassKernelResults of the most recent run (for harnesses)


# ---------------------------------------------------------------- planning

def _pack(profile, units):
    """Greedy: place unit needs into 8x profile slot inventory, splitting
    freely; returns list of (unit, off, cnt, size_class) pieces or None."""
    inv = []
    for s in profile:
        inv += [s] * NCORES
    inv.sort(reverse=True)
    pieces = []
    order = sorted(range(len(units)), key=lambda o: -units[o][1])
    for o in order:
        u, n = units[o]
        rem, off = n, 0
        while rem > 0:
            if not inv:
                return None
            le = [s for s in inv if s <= rem]
            s = max(le) if le else min(inv)
            inv.remove(s)
            take = min(s, rem)
            pieces.append((u, off, take, s))
            off += take
            rem -= take
    return pieces


def _best_profile(units):
    """units: list of (unit_idx, need).  Returns (profile, pieces)."""
    if not units:
        return (), []
    total = sum(n for _, n in units)
    tmin = (total + NCORES - 1) // NCORES
    from itertools import combinations_with_replacement as cwr
    best = None
    for T in range(tmin, tmin + 4):
        for m in range(1, 6):
            for prof in cwr(range(1, KT + 1), m):
                if sum(prof) != T:
                    continue
                pieces = _pack(prof, units)
                if pieces is None:
                    continue
                key = (T, m)
                if best is None or key < best[0]:
                    best = (key, tuple(sorted(prof, reverse=True)), pieces)
        if best is not None and best[0][0] == T:
            break
    return best[1], best[2]


def _plan(need, fp8_unit):
    """Returns (slots, cls8, assign): slot sizes in execution order,
    per-slot fp8 flag, and assign[core][slot] = (unit|None, off, cnt)."""
    units8 = [(u, n) for u, n in enumerate(need) if fp8_unit[u]]
    units16 = [(u, n) for u, n in enumerate(need) if not fp8_unit[u]]
    prof8, pieces8 = _best_profile(units8)
    prof16, pieces16 = _best_profile(units16)
    # execution order: all slots ascending by size, smallest last (tail)
    tagged = [(s, True) for s in prof8] + [(s, False) for s in prof16]
    tagged.sort(key=lambda x: x[0])
    order = tagged[1:] + [tagged[0]] if len(tagged) >= 2 else tagged
    slots = tuple(s for s, _ in order)
    cls8 = tuple(c for _, c in order)
    cells = {}   # (size, is8) -> list of (core, slot_idx)
    for j, (s, c) in enumerate(zip(slots, cls8)):
        cells.setdefault((s, c), [])
        for co in range(NCORES):
            cells[(s, c)].append((co, j))
    assign = [[None] * len(slots) for _ in range(NCORES)]
    for pieces, is8 in ((pieces8, True), (pieces16, False)):
        for (u, off, cnt, s) in pieces:
            co, j = cells[(s, is8)].pop()
            assign[co][j] = (u, off, cnt)
    for co in range(NCORES):
        for j in range(len(slots)):
            if assign[co][j] is None:
                assign[co][j] = (None, 0, 0)
    return slots, cls8, assign


def _ownership(slots):
    """Per slot: which exp tiles run on DVE (True) vs ACT (False).
    The final (tail) slot is DVE so its exp chains with the drain copy."""
    T = sum(slots)
    dve_total = T - ACT_TILES
    owner = []
    for j, t in enumerate(slots):
        if j == len(slots) - 1:
            owner.append(tuple(True for _ in range(t)))
            continue
        d = int(round(t * dve_total / max(T, 1)))
        d = min(t, max(0, d))
        dve_pos = set()
        if d > 0:
            for i in range(d):
                dve_pos.add(int((i + 0.5) * t / d))
        owner.append(tuple(i in dve_pos for i in range(t)))
    return tuple(owner)


# ---------------------------------------------------------------- build

def _build(slots, cls8, owner):
    nc = bacc.Bacc(num_devices=NCORES)
    NS = len(slots)
    T = sum(slots)
    npairs = [(t + 1) // 2 for t in slots]
    soff = [sum(slots[:j]) for j in range(NS)]

    kqs = [
        nc.declare_dram_parameter(f"kq{j}", [128, slots[j] * 128 + QCH], F16,
                                  isOutput=False)
        for j in range(NS)
    ]
    vs = [
        nc.declare_dram_parameter(f"v{j}", [128, npairs[j] * 2, 128],
                                  F8 if cls8[j] else F16, isOutput=False)
        for j in range(NS)
    ]
    # per-slot cols [4*soff, 4*soff+4*t): [sc_a | bi_a | sc_d | bi_d]
    sb = nc.declare_dram_parameter("sb", [128, 4 * T], F32, isOutput=False)
    oT = nc.declare_dram_parameter("oT", [NS, D, QCH], F16, isOutput=True)
    n8 = sum(npairs[j] for j in range(NS) if cls8[j])
    n16 = sum(npairs[j] for j in range(NS) if not cls8[j])
    ep8 = nc.declare_dram_parameter("ep8", [max(n8, 1), 128, 2, QCH], F8,
                                    isOutput=True)
    ep16 = nc.declare_dram_parameter("ep16", [max(n16, 1), 128, 2, QCH], F16,
                                     isOutput=True)

    with tile.TileContext(nc) as tc:
        with (
            tc.tile_pool(name="consts", bufs=1) as consts,
            tc.tile_pool(name="inputs", bufs=4) as inpool,
            tc.tile_pool(name="epool", bufs=max(npairs) + 4) as epool,
            tc.tile_pool(name="osb", bufs=2) as opool,
            tc.tile_pool(name="sps", bufs=3, space="PSUM") as pspool,
            tc.tile_pool(name="oacc", bufs=1, space="PSUM") as psacc,
        ):
            sb_sb = consts.tile([128, 4 * T], F32)

            # --- HAM warm-up: dummy bf16 matmuls while input DMAs stream ---
            wsrc = consts.tile([128, MM_N], BF16)
            nc.vector.memset(wsrc[:], 1.0)
            for w in range(WARMUP_MMS):
                if w % 2 == 0:
                    wps = pspool.tile([128, QCH], F32, tag="s")
                nc.tensor.matmul(
                    wps[:, (w % 2) * MM_N : (w % 2) * MM_N + MM_N],
                    wsrc[:, :128],
                    wsrc[:],
                    start=True,
                    stop=True,
                    skip_group_check=True,
                )

            ncopy = [0]
            i8 = [0]
            i16 = [0]
            for s in range(NS):
                t = slots[s]
                np_s = npairs[s]
                is8 = cls8[s]
                ow = owner[s]
                edt = F8 if is8 else F16
                kq_sb = inpool.tile([128, t * 128 + QCH], F16, tag="kq")
                v_sb = inpool.tile([128, np_s * 2, 128], edt, tag="v")
                # one packed kt|qT kick + one v kick per slot, all on the SP
                # queue so program order = priority order
                if s == 0:
                    # split so the first S matmul can start on the first piece
                    half = t * 128 + QCH // 2
                    nc.sync.dma_start(out=kq_sb[:, :half], in_=kqs[s][:, :half])
                    nc.sync.dma_start(out=kq_sb[:, half:], in_=kqs[s][:, half:])
                else:
                    nc.sync.dma_start(out=kq_sb[:, :], in_=kqs[s][:, :])
                nc.sync.dma_start(out=v_sb[:, :, :], in_=vs[s][:, :, :])
                if s == 0:
                    # masks land after slot0's inputs, before the first exp
                    nc.sync.dma_start(out=sb_sb[:], in_=sb[:])
                kt_sb = kq_sb[:, : t * 128]
                qT_sb = kq_sb[:, t * 128 :]

                o_ps = psacc.tile([128, QCH], F32, tag="o")
                epair = None
                for i in range(t):
                    m = i // 2
                    if i % 2 == 0:
                        epair = epool.tile([128, 2, QCH], edt, tag="e")
                    s_ps = pspool.tile([128, QCH], F32, tag="s")
                    for h in range(QCH // MM_N):
                        nc.tensor.matmul(
                            s_ps[:, bass.ts(h, MM_N)],
                            kt_sb[:, bass.ts(i, 128)],
                            qT_sb[:, bass.ts(h, MM_N)],
                            start=True,
                            stop=True,
                        )
                    col = 4 * soff[s]
                    if ow[i]:
                        sc_ap = sb_sb[:, col + 2 * t + i : col + 2 * t + i + 1]
                        bi_ap = sb_sb[:, col + 3 * t + i : col + 3 * t + i + 1]
                        nc.vector.tensor_scalar(
                            epair[:, i % 2, :].bitcast(U8 if is8 else I16),
                            s_ps[:], sc_ap, bi_ap,
                            mybir.AluOpType.mult, mybir.AluOpType.add,
                        )
                    else:
                        sc_ap = sb_sb[:, col + i : col + i + 1]
                        bi_ap = sb_sb[:, col + t + i : col + t + i + 1]
                        parts = (
                            [bass.ts(p, MM_N) for p in range(2)]
                            if (s == 0 and i == 0)
                            else [slice(None)]
                        )
                        for pr in parts:
                            nc.scalar.activation(
                                epair[:, i % 2, pr],
                                s_ps[:, pr],
                                mybir.ActivationFunctionType.Exp,
                                bias=bi_ap,
                                scale=sc_ap,
                            )
                    last = i == t - 1
                    if is8 and i % 2 == 1:
                        # full fp8 pair -> DoubleRow matmul (2 key tiles/pass)
                        for h in range(QCH // MM_N):
                            nc.tensor.matmul(
                                o_ps[:, bass.ts(h, MM_N)],
                                v_sb[:, 2 * m : 2 * m + 2, :],
                                epair[:, :, bass.ts(h, MM_N)],
                                start=(m == 0),
                                stop=last,
                                perf_mode=mybir.MatmulPerfMode.DoubleRow,
                            )
                    elif is8 and last:
                        # odd fp8 tail: single regular fp8 matmul
                        for h in range(QCH // MM_N):
                            nc.tensor.matmul(
                                o_ps[:, bass.ts(h, MM_N)],
                                v_sb[:, 2 * m, :],
                                epair[:, 0, bass.ts(h, MM_N)],
                                start=(i == 0),
                                stop=True,
                            )
                    elif not is8:
                        # fp16: regular matmul per key tile
                        for h in range(QCH // MM_N):
                            nc.tensor.matmul(
                                o_ps[:, bass.ts(h, MM_N)],
                                v_sb[:, i, :],
                                epair[:, i % 2, bass.ts(h, MM_N)],
                                start=(i == 0),
                                stop=last,
                            )
                    if i % 2 == 1 or last:
                        # odd tail pair: only the lower half holds data
                        hs = 1 if (last and t % 2 == 1) else 2
                        if is8:
                            nc.gpsimd.dma_start(out=ep8[i8[0]][:, :hs, :],
                                                in_=epair[:, :hs, :])
                            i8[0] += 1
                        else:
                            nc.gpsimd.dma_start(out=ep16[i16[0]][:, :hs, :],
                                                in_=epair[:, :hs, :])
                            i16[0] += 1

                o_sb = opool.tile([128, QCH], F16, tag="osb")
                if ncopy[0] < ACT_COPIES:
                    nc.scalar.copy(o_sb[:], o_ps[:])
                else:
                    nc.vector.tensor_copy(o_sb[:], o_ps[:])
                ncopy[0] += 1
                nc.gpsimd.dma_start(out=oT[s], in_=o_sb[:])

    nc.compile()
    return nc


# ---------------------------------------------------------------- host

def kernel(q, k, v, valid_lens):
    q = np.ascontiguousarray(q, dtype=np.float32)
    k = np.ascontiguousarray(k, dtype=np.float32)
    v = np.ascontiguousarray(v, dtype=np.float32)
    L = np.asarray(valid_lens).reshape(-1).astype(np.int64)

    need_b = np.where(L == 0, KT, np.minimum(KT, (L + 127) // 128)).astype(np.int64)
    units = [(b, h) for b in range(B) for h in range(Q // QCH)]
    need = [int(need_b[b]) for b, h in units]
    fp8_unit = [int(L[b]) >= L8 for b, h in units]

    slots, cls8, assign = _plan(need, fp8_unit)
    owner = _ownership(slots)

    key = (slots, cls8, owner)
    if key not in _build_cache:
        _build_cache[key] = _build(slots, cls8, owner)
    nc = _build_cache[key]

    qh = q.astype(np.float16)
    kh = k.astype(np.float16)
    vh = v.astype(np.float16)
    v8 = v.astype(FN)

    NS = len(slots)
    T = sum(slots)
    npairs = [(t + 1) // 2 for t in slots]
    soff = [sum(slots[:j]) for j in range(NS)]

    in_maps = []
    for c in range(NCORES):
        sb_arr = np.zeros((128, 4 * T), np.float32)
        im = {"sb": sb_arr}
        for j in range(NS):
            t = slots[j]
            is8 = cls8[j]
            kq = np.zeros((128, t * 128 + QCH), np.float16)
            vv = np.zeros((128, npairs[j] * 2, 128), FN if is8 else np.float16)
            u, off, cnt = assign[c][j]
            col = 4 * soff[j]
            if u is not None and cnt > 0:
                b, h = units[u]
                lb = int(L[b])
                kq[:, t * 128 :] = qh[b, h * QCH : (h + 1) * QCH].T
                kq[:, : cnt * 128] = kh[b, off * 128 : (off + cnt) * 128].T
                vsl = (v8 if is8 else vh)[b, off * 128 : (off + cnt) * 128]
                for i in range(cnt):
                    vv[:, i, :] = vsl[i * 128 : (i + 1) * 128]
                kidx = np.arange(128)
                smul = SCH8_MUL if is8 else SCH_MUL
                sbia = SCH8_BIAS if is8 else SCH_BIAS
                one = 56.0 if is8 else 15360.0
                for i in range(cnt):
                    base = (off + i) * 128
                    if lb == 0:
                        sb_arr[:, col + 3 * t + i] = one   # E = 1.0 uniform
                    else:
                        m = (base + kidx < lb).astype(np.float32)
                        sb_arr[:, col + i] = m * np.float32(SCALE)
                        sb_arr[:, col + t + i] = np.where(
                            m > 0, np.float32(-SHIFT), np.float32(NEG_BIAS))
                        sb_arr[:, col + 2 * t + i] = m * np.float32(SCALE * smul)
                        sb_arr[:, col + 3 * t + i] = m * np.float32(sbia)
            for i in range(cnt, t):
                sb_arr[:, col + t + i] = np.float32(NEG_BIAS)  # ACT padding
                # DVE padding: sc=0, bi=0 -> +0.0
            im[f"kq{j}"] = kq
            im[f"v{j}"] = vv
        in_maps.append(im)

    res = run_bass_kernel_spmd(nc, in_maps, list(range(NCORES)))
    global last_result
    last_result = res

    num = [np.zeros((D, QCH), np.float32) for _ in range(len(units))]
    den = [np.zeros((QCH,), np.float32) for _ in range(len(units))]
    # replicate the build's ep8/ep16 stream ordering: (slot, pair) -> index
    pair_idx = {}
    c8 = c16 = 0
    for j in range(NS):
        for m in range(npairs[j]):
            if cls8[j]:
                pair_idx[(j, m)] = (True, c8)
                c8 += 1
            else:
                pair_idx[(j, m)] = (False, c16)
                c16 += 1
    for c in range(NCORES):
        r = res.results[c]
        e8v = r["ep8"]
        e8v = (e8v.view(FN) if e8v.dtype != FN else e8v).astype(np.float32)
        e16v = r["ep16"].astype(np.float32)
        for j in range(NS):
            u, off, cnt = assign[c][j]
            if u is None or cnt == 0:
                continue
            num[u] += r["oT"][j].astype(np.float32)
            for i in range(cnt):
                isf8, idx = pair_idx[(j, i // 2)]
                arr = e8v if isf8 else e16v
                den[u] += arr[idx, :, i % 2, :].sum(axis=0)

    out = np.empty((B, Q, D), np.float32)
    for ui, (b, h) in enumerate(units):
        out[b, h * QCH : (h + 1) * QCH] = (num[ui] / den[ui][None, :]).T
    return out


# revision 18
# speedup vs baseline: 1.1521x; 1.0375x over previous
"""Dense dot-product attention with key-length masking on 8 Trainium2 cores.

Problem: q,k,v [16, 2048, 128] fp32, valid_lens [16,1] int32.
  out = softmax(mask(q@k.T/sqrt(d))) @ v   (masked keys -> -1e6 before softmax)

v4 design:
- Flexible work packing: every core runs the same slot-size profile; a
  slot holds ONE (batch, q-half) unit's key-tile RANGE, and a unit may
  split across any slots/cores.  The host adds partial numerators /
  denominators, so packing is near perfect; per-tile masks make padding
  free.
- Slots come in TWO dtype classes, baked into the program:
  * fp8 slots hold tiles of LARGE-valid_len units (L >= 1024).  Their
    softmax is diffuse (max weight ~1-2%), so e4m3's 6% quantization is
    harmless.  E pairs [128,2,1024] fp8 drive DoubleRow matmuls (2 key
    tiles per pass = 2x PE).
  * fp16 slots hold small-L units (peaked softmax needs precision).
    Regular fp16 matmuls.
- E = exp(S/sqrt(d) - 2): the -2 shift cancels in softmax and keeps
  E < 240 (e4m3-safe for scores up to 7.5 sigma).
- exp runs on TWO engines: ACT (activation Exp) and DVE (Schraudolph
  bit trick: round(S*a + b) in uint8/int16, bitcast as e4m3/fp16;
  uint8/int16 conversion saturates underflow to +0.0; masked partitions
  get scale'=0, bias'=0 -> E=+0.0).
- NO on-device softmax reduction: all E pairs stream to DRAM and the
  host computes denominators from exactly the bytes the PE consumed.
- oT fp32->fp16 PSUM drain copies split between ACT and DVE.
- Input DMA on the SP queue in slot order; outputs kick on the idle
  Pool queue (Pool tensor math would contend for SBUF ports with DVE).
- HAM warm-up: dummy bf16 matmuls while the input DMAs stream.
"""

import math
import sys
import types

import numpy as np
import ml_dtypes

import concourse.bass as bass
import concourse.mybir as mybir
import concourse.tile as tile
from concourse.tile import add_dep_helper
from concourse import bacc
from concourse.bass_utils import run_bass_kernel_spmd

B, Q, K, D = 16, 2048, 2048, 128
NCORES = 8
QCH = 1024         # queries per work unit
KT = K // 128      # max key tiles per unit
MM_N = 512         # moving-operand free dim per matmul
SCALE = 1.0 / math.sqrt(D)
SHIFT = 2.0        # exp(S*SCALE - SHIFT): cancels in softmax, keeps E < 240
L8 = 1024          # units with L >= L8 use fp8 E (diffuse softmax)
NEG_BIAS = -30.0   # exp(-30) ~ 1e-13 -> flushes to +0
WARMUP_MMS = 5     # dummy matmuls to lift the PE HAM clock-gate
LOG2E = math.log2(math.e)
SCH_MUL = 1024.0 * LOG2E                             # fp16 Schraudolph
SCH_BIAS = 15360.0 - 1024.0 * LOG2E * SHIFT - 45.0
SCH8_MUL = 8.0 * LOG2E                               # e4m3 Schraudolph
SCH8_BIAS = 56.0 - 8.0 * LOG2E * SHIFT - 0.351
ACT_TILES = 18                    # exp tiles on ACT (rest on DVE)
ACT_COPIES = 3                    # oT drain copies done by ACT (rest DVE)

F32 = mybir.dt.float32
F16 = mybir.dt.float16
BF16 = mybir.dt.bfloat16
F8 = mybir.dt.float8e4
U8 = mybir.dt.uint8
I16 = mybir.dt.int16
FN = ml_dtypes.float8_e4m3fn


def _install_hook_stub():
    """bass_utils' axon trace path imports antenv.axon_hooks, which is not
    shipped in this container.  Provide a no-op stub so an ambient
    BASS_TRACE=1 doesn't crash; test harnesses may overwrite the hook."""
    if "antenv.axon_hooks" in sys.modules:
        return
    mod = types.ModuleType("antenv.axon_hooks")
    _hook = [None]
    mod.set_axon_ntff_profile_hook = lambda h: _hook.__setitem__(0, h)
    mod.get_axon_ntff_profile_hook = lambda: _hook[0]
    sys.modules["antenv.axon_hooks"] = mod


_install_hook_stub()

_build_cache = {}
last_result = None  # BassKernelResults of the most recent run (for harnesses)


# ---------------------------------------------------------------- planning

def _pack(profile, units):
    """Greedy: place unit needs into 8x profile slot inventory, splitting
    freely; returns list of (unit, off, cnt, size_class) pieces or None."""
    inv = []
    for s in profile:
        inv += [s] * NCORES
    inv.sort(reverse=True)
    pieces = []
    order = sorted(range(len(units)), key=lambda o: -units[o][1])
    for o in order:
        u, n = units[o]
        rem, off = n, 0
        while rem > 0:
            if not inv:
                return None
            le = [s for s in inv if s <= rem]
            s = max(le) if le else min(inv)
            inv.remove(s)
            take = min(s, rem)
            pieces.append((u, off, take, s))
            off += take
            rem -= take
    return pieces


def _best_profile(units):
    """units: list of (unit_idx, need).  Returns (profile, pieces)."""
    if not units:
        return (), []
    total = sum(n for _, n in units)
    tmin = (total + NCORES - 1) // NCORES
    from itertools import combinations_with_replacement as cwr
    best = None
    for T in range(tmin, tmin + 4):
        for m in range(1, 6):
            for prof in cwr(range(1, KT + 1), m):
                if sum(prof) != T:
                    continue
                pieces = _pack(prof, units)
                if pieces is None:
                    continue
                key = (T, m)
                if best is None or key < best[0]:
                    best = (key, tuple(sorted(prof, reverse=True)), pieces)
        if best is not None and best[0][0] == T:
            break
    return best[1], best[2]


def _plan(need, fp8_unit):
    """Returns (slots, cls8, assign): slot sizes in execution order,
    per-slot fp8 flag, and assign[core][slot] = (unit|None, off, cnt)."""
    units8 = [(u, n) for u, n in enumerate(need) if fp8_unit[u]]
    units16 = [(u, n) for u, n in enumerate(need) if not fp8_unit[u]]
    prof8, pieces8 = _best_profile(units8)
    prof16, pieces16 = _best_profile(units16)
    # execution order: ramp up (small -> large) so each slot's compute
    # covers the next slot's input stream; the two smallest slots last
    # (small tail, and their inputs are long since arrived)
    tagged = [(s, True) for s in prof8] + [(s, False) for s in prof16]
    tagged.sort(key=lambda x: x[0])
    if len(tagged) > 2:
        order = tagged[2:] + [tagged[1], tagged[0]]
    else:
        order = tagged
    slots = tuple(s for s, _ in order)
    cls8 = tuple(c for _, c in order)
    cells = {}   # (size, is8) -> list of (core, slot_idx)
    for j, (s, c) in enumerate(zip(slots, cls8)):
        cells.setdefault((s, c), [])
        for co in range(NCORES):
            cells[(s, c)].append((co, j))
    assign = [[None] * len(slots) for _ in range(NCORES)]
    for pieces, is8 in ((pieces8, True), (pieces16, False)):
        for (u, off, cnt, s) in pieces:
            co, j = cells[(s, is8)].pop()
            assign[co][j] = (u, off, cnt)
    for co in range(NCORES):
        for j in range(len(slots)):
            if assign[co][j] is None:
                assign[co][j] = (None, 0, 0)
    return slots, cls8, assign


def _ownership(slots):
    """Per slot: which exp tiles run on DVE (True) vs ACT (False).
    The final (tail) slot is DVE so its exp chains with the drain copy."""
    T = sum(slots)
    dve_total = T - ACT_TILES
    owner = []
    for j, t in enumerate(slots):
        if j == len(slots) - 1:
            owner.append(tuple(True for _ in range(t)))
            continue
        d = int(round(t * dve_total / max(T, 1)))
        d = min(t, max(0, d))
        dve_pos = set()
        if d > 0:
            for i in range(d):
                dve_pos.add(int((i + 0.5) * t / d))
        owner.append(tuple(i in dve_pos for i in range(t)))
    return tuple(owner)


# ---------------------------------------------------------------- build

def _build(slots, cls8, owner):
    nc = bacc.Bacc(num_devices=NCORES)
    NS = len(slots)
    T = sum(slots)
    npairs = [(t + 1) // 2 for t in slots]
    soff = [sum(slots[:j]) for j in range(NS)]

    kqs = [
        nc.declare_dram_parameter(f"kq{j}", [128, slots[j] * 128 + QCH], F16,
                                  isOutput=False)
        for j in range(NS)
    ]
    vs = [
        nc.declare_dram_parameter(f"v{j}", [128, npairs[j] * 2, 128],
                                  F8 if cls8[j] else F16, isOutput=False)
        for j in range(NS)
    ]
    # per-slot cols [4*soff, 4*soff+4*t): [sc_a | bi_a | sc_d | bi_d]
    sb = nc.declare_dram_parameter("sb", [128, 4 * T], F32, isOutput=False)
    oT = nc.declare_dram_parameter("oT", [NS, D, QCH], F16, isOutput=True)
    n8 = sum(npairs[j] for j in range(NS) if cls8[j])
    n16 = sum(npairs[j] for j in range(NS) if not cls8[j])
    ep8 = nc.declare_dram_parameter("ep8", [max(n8, 1), 128, 2, QCH], F8,
                                    isOutput=True)
    ep16 = nc.declare_dram_parameter("ep16", [max(n16, 1), 128, 2, QCH], F16,
                                     isOutput=True)

    with tile.TileContext(nc) as tc:
        with (
            tc.tile_pool(name="consts", bufs=1) as consts,
            tc.tile_pool(name="inputs", bufs=4) as inpool,
            tc.tile_pool(name="epool", bufs=max(npairs) + 8) as epool,
            tc.tile_pool(name="osb", bufs=2) as opool,
            tc.tile_pool(name="sps", bufs=3, space="PSUM") as pspool,
            tc.tile_pool(name="oacc", bufs=1, space="PSUM") as psacc,
        ):
            sb_sb = consts.tile([128, 4 * T], F32)

            # --- HAM warm-up: dummy bf16 matmuls while input DMAs stream ---
            wsrc = consts.tile([128, MM_N], BF16)
            nc.vector.memset(wsrc[:], 1.0)
            for w in range(WARMUP_MMS):
                if w % 2 == 0:
                    wps = pspool.tile([128, QCH], F32, tag="s")
                nc.tensor.matmul(
                    wps[:, (w % 2) * MM_N : (w % 2) * MM_N + MM_N],
                    wsrc[:, :128],
                    wsrc[:],
                    start=True,
                    stop=True,
                    skip_group_check=True,
                )

            ncopy = [0]
            i8 = [0]
            i16 = [0]
            for s in range(NS):
                t = slots[s]
                np_s = npairs[s]
                is8 = cls8[s]
                ow = owner[s]
                edt = F8 if is8 else F16
                kq_sb = inpool.tile([128, t * 128 + QCH], F16, tag="kq")
                v_sb = inpool.tile([128, np_s * 2, 128], edt, tag="v")
                # one packed kt|qT kick + one v kick per slot, all on the SP
                # queue so program order = priority order
                if s == 0:
                    # split so the first S matmul can start on the first piece
                    half = t * 128 + QCH // 2
                    nc.sync.dma_start(out=kq_sb[:, :half], in_=kqs[s][:, :half])
                    nc.sync.dma_start(out=kq_sb[:, half:], in_=kqs[s][:, half:])
                else:
                    nc.sync.dma_start(out=kq_sb[:, :], in_=kqs[s][:, :])
                nc.sync.dma_start(out=v_sb[:, :, :], in_=vs[s][:, :, :])
                if s == 0:
                    # masks land after slot0's inputs, before the first exp
                    nc.sync.dma_start(out=sb_sb[:], in_=sb[:])
                kt_sb = kq_sb[:, : t * 128]
                qT_sb = kq_sb[:, t * 128 :]

                o_ps = psacc.tile([128, QCH], F32, tag="o")
                epair = None
                for i in range(t):
                    m = i // 2
                    if i % 2 == 0:
                        epair = epool.tile([128, 2, QCH], edt, tag="e")
                    s_ps = pspool.tile([128, QCH], F32, tag="s")
                    for h in range(QCH // MM_N):
                        nc.tensor.matmul(
                            s_ps[:, bass.ts(h, MM_N)],
                            kt_sb[:, bass.ts(i, 128)],
                            qT_sb[:, bass.ts(h, MM_N)],
                            start=True,
                            stop=True,
                        )
                    col = 4 * soff[s]
                    if ow[i]:
                        sc_ap = sb_sb[:, col + 2 * t + i : col + 2 * t + i + 1]
                        bi_ap = sb_sb[:, col + 3 * t + i : col + 3 * t + i + 1]
                        nc.vector.tensor_scalar(
                            epair[:, i % 2, :].bitcast(U8 if is8 else I16),
                            s_ps[:], sc_ap, bi_ap,
                            mybir.AluOpType.mult, mybir.AluOpType.add,
                        )
                    else:
                        sc_ap = sb_sb[:, col + i : col + i + 1]
                        bi_ap = sb_sb[:, col + t + i : col + t + i + 1]
                        parts = (
                            [bass.ts(p, MM_N) for p in range(2)]
                            if (s == 0 and i == 0)
                            else [slice(None)]
                        )
                        for pr in parts:
                            nc.scalar.activation(
                                epair[:, i % 2, pr],
                                s_ps[:, pr],
                                mybir.ActivationFunctionType.Exp,
                                bias=bi_ap,
                                scale=sc_ap,
                            )
                    last = i == t - 1
                    if is8 and i % 2 == 1:
                        # full fp8 pair -> DoubleRow matmul (2 key tiles/pass)
                        for h in range(QCH // MM_N):
                            nc.tensor.matmul(
                                o_ps[:, bass.ts(h, MM_N)],
                                v_sb[:, 2 * m : 2 * m + 2, :],
                                epair[:, :, bass.ts(h, MM_N)],
                                start=(m == 0),
                                stop=last,
                                perf_mode=mybir.MatmulPerfMode.DoubleRow,
                            )
                    elif is8 and last:
                        # odd fp8 tail: single regular fp8 matmul
                        for h in range(QCH // MM_N):
                            nc.tensor.matmul(
                                o_ps[:, bass.ts(h, MM_N)],
                                v_sb[:, 2 * m, :],
                                epair[:, 0, bass.ts(h, MM_N)],
                                start=(i == 0),
                                stop=True,
                            )
                    elif not is8:
                        # fp16: regular matmul per key tile
                        for h in range(QCH // MM_N):
                            nc.tensor.matmul(
                                o_ps[:, bass.ts(h, MM_N)],
                                v_sb[:, i, :],
                                epair[:, i % 2, bass.ts(h, MM_N)],
                                start=(i == 0),
                                stop=last,
                            )
                    if i % 2 == 1 or last:
                        # odd tail pair: only the lower half holds data
                        hs = 1 if (last and t % 2 == 1) else 2
                        if is8:
                            nc.gpsimd.dma_start(out=ep8[i8[0]][:, :hs, :],
                                                in_=epair[:, :hs, :])
                            i8[0] += 1
                        else:
                            nc.gpsimd.dma_start(out=ep16[i16[0]][:, :hs, :],
                                                in_=epair[:, :hs, :])
                            i16[0] += 1

                o_sb = opool.tile([128, QCH], F16, tag="osb")
                if ncopy[0] < ACT_COPIES:
                    nc.scalar.copy(o_sb[:], o_ps[:])
                else:
                    nc.vector.tensor_copy(o_sb[:], o_ps[:])
                ncopy[0] += 1
                nc.gpsimd.dma_start(out=oT[s], in_=o_sb[:])

    nc.compile()
    return nc


# ---------------------------------------------------------------- host

def kernel(q, k, v, valid_lens):
    q = np.ascontiguousarray(q, dtype=np.float32)
    k = np.ascontiguousarray(k, dtype=np.float32)
    v = np.ascontiguousarray(v, dtype=np.float32)
    L = np.asarray(valid_lens).reshape(-1).astype(np.int64)

    need_b = np.where(L == 0, KT, np.minimum(KT, (L + 127) // 128)).astype(np.int64)
    units = [(b, h) for b in range(B) for h in range(Q // QCH)]
    need = [int(need_b[b]) for b, h in units]
    fp8_unit = [int(L[b]) >= L8 for b, h in units]

    slots, cls8, assign = _plan(need, fp8_unit)
    owner = _ownership(slots)

    key = (slots, cls8, owner)
    if key not in _build_cache:
        _build_cache[key] = _build(slots, cls8, owner)
    nc = _build_cache[key]

    qh = q.astype(np.float16)
    kh = k.astype(np.float16)
    vh = v.astype(np.float16)
    v8 = v.astype(FN)

    NS = len(slots)
    T = sum(slots)
    npairs = [(t + 1) // 2 for t in slots]
    soff = [sum(slots[:j]) for j in range(NS)]

    in_maps = []
    for c in range(NCORES):
        sb_arr = np.zeros((128, 4 * T), np.float32)
        im = {"sb": sb_arr}
        for j in range(NS):
            t = slots[j]
            is8 = cls8[j]
            kq = np.zeros((128, t * 128 + QCH), np.float16)
            vv = np.zeros((128, npairs[j] * 2, 128), FN if is8 else np.float16)
            u, off, cnt = assign[c][j]
            col = 4 * soff[j]
            if u is not None and cnt > 0:
                b, h = units[u]
                lb = int(L[b])
                kq[:, t * 128 :] = qh[b, h * QCH : (h + 1) * QCH].T
                kq[:, : cnt * 128] = kh[b, off * 128 : (off + cnt) * 128].T
                vsl = (v8 if is8 else vh)[b, off * 128 : (off + cnt) * 128]
                for i in range(cnt):
                    vv[:, i, :] = vsl[i * 128 : (i + 1) * 128]
                kidx = np.arange(128)
                smul = SCH8_MUL if is8 else SCH_MUL
                sbia = SCH8_BIAS if is8 else SCH_BIAS
                one = 56.0 if is8 else 15360.0
                for i in range(cnt):
                    base = (off + i) * 128
                    if lb == 0:
                        sb_arr[:, col + 3 * t + i] = one   # E = 1.0 uniform
                    else:
                        m = (base + kidx < lb).astype(np.float32)
                        sb_arr[:, col + i] = m * np.float32(SCALE)
                        sb_arr[:, col + t + i] = np.where(
                            m > 0, np.float32(-SHIFT), np.float32(NEG_BIAS))
                        sb_arr[:, col + 2 * t + i] = m * np.float32(SCALE * smul)
                        sb_arr[:, col + 3 * t + i] = m * np.float32(sbia)
            for i in range(cnt, t):
                sb_arr[:, col + t + i] = np.float32(NEG_BIAS)  # ACT padding
                # DVE padding: sc=0, bi=0 -> +0.0
            im[f"kq{j}"] = kq
            im[f"v{j}"] = vv
        in_maps.append(im)

    res = run_bass_kernel_spmd(nc, in_maps, list(range(NCORES)))
    global last_result
    last_result = res

    num = [np.zeros((D, QCH), np.float32) for _ in range(len(units))]
    den = [np.zeros((QCH,), np.float32) for _ in range(len(units))]
    # replicate the build's ep8/ep16 stream ordering: (slot, pair) -> index
    pair_idx = {}
    c8 = c16 = 0
    for j in range(NS):
        for m in range(npairs[j]):
            if cls8[j]:
                pair_idx[(j, m)] = (True, c8)
                c8 += 1
            else:
                pair_idx[(j, m)] = (False, c16)
                c16 += 1
    for c in range(NCORES):
        r = res.results[c]
        e8v = r["ep8"]
        e8v = (e8v.view(FN) if e8v.dtype != FN else e8v).astype(np.float32)
        e16v = r["ep16"].astype(np.float32)
        for j in range(NS):
            u, off, cnt = assign[c][j]
            if u is None or cnt == 0:
                continue
            num[u] += r["oT"][j].astype(np.float32)
            for i in range(cnt):
                isf8, idx = pair_idx[(j, i // 2)]
                arr = e8v if isf8 else e16v
                den[u] += arr[idx, :, i % 2, :].sum(axis=0)

    out = np.empty((B, Q, D), np.float32)
    for ui, (b, h) in enumerate(units):
        out[b, h * QCH : (h + 1) * QCH] = (num[ui] / den[ui][None, :]).T
    return out
